# revision 20
# baseline (speedup 1.0000x reference)
"""BinaryConv2d Trainium2 kernel (8-core batch-parallel, fp8 DoubleRow PE).

Per image: top half rows on partitions 0:64, bottom half on 64:128 (each a
zero-padded fp8 slab of HALF+2 rows; sign(x) is exactly representable in
fp8e4). Each 3x3 conv runs as FIVE DoubleRow matmuls per PSUM tile (two
kernel positions contracted per matmul, 2 fp8 elems/partition/cycle), issued
concurrently on PE row groups (tile_position (0,0) and (64,0)). Output is
scaled on DVE into bf16 (integer counts <=576 keep full precision at the
needed tolerance) halving output DMA; host upcasts to fp32.
"""
import sys
import numpy as np
from contextlib import ExitStack

sys.path.insert(0, "/root/.axon_site/_ro/trn_rl_repo")
sys.path.insert(0, "/opt/trn_rl_repo")

import ml_dtypes
import concourse.bass as bass
import concourse.bacc as bacc
import concourse.mybir as mybir
import concourse.tile as tile
from concourse.ap import AP
from concourse.bass_utils import run_bass_kernel_spmd

F32 = mybir.dt.float32
BF16 = mybir.dt.bfloat16
FP8 = mybir.dt.float8e4
# A DoubleRow matmul with start=False,stop=False (middle of an accumulation
# group) hard-faults the exec unit, and DoubleRowSwInterleave fails walrus's
# ldweights ISA check. So DR pairs are legal only as the start and stop
# matmuls of a group; the other 5 taps run as regular-fp8 singles.
DR = mybir.MatmulPerfMode.DoubleRow

N_CORES = 8
B, CIN, COUT, KS = 32, 64, 64, 3
H = W = 160
B_CORE = B // N_CORES
HALF = H // 2          # rows per half
SH = HALF + 2          # slab rows per half (1 halo/pad row each side)
PW = W + 2
RPT = 3                # output rows per PSUM tile

# Per-PSUM-tile schedule: 7 matmuls. DR pairs ((0,0),(1,0)) and ((0,2),(1,2))
# at the start/stop slots; taps (0,1),(1,1),(2,1),(2,0),(2,2) as regular-fp8
# singles in between. Entries: (weight offset blocks, (row, col) or
# (row, col, pairstride) for DR).
# Weight free-dim layout (64-wide blocks):
#   [0]=(0,0) [1]=(1,0) [2]=(0,1) [3]=(1,1) [4]=(2,1) [5]=(2,0) [6]=(2,2)
#   [7]=(0,2) [8]=(1,2)
SCHED = [
    ("dr", 0, 0, 0),     # pair blocks 0,1: taps (0,0),(1,0); mov base (h0, 0)
    ("s", 2, 0, 1),      # (0,1)
    ("s", 3, 1, 1),      # (1,1)
    ("s", 4, 2, 1),      # (2,1)
    ("s", 5, 2, 0),      # (2,0)
    ("s", 6, 2, 2),      # (2,2)
    ("dr", 7, 0, 2),     # pair blocks 7,8: taps (0,2),(1,2); mov base (h0, 2)
]
WBLOCKS = [
    (0, 0), (1, 0), (0, 1), (1, 1), (2, 1), (2, 0), (2, 2), (0, 2), (1, 2)
]


def build_nc(n_img=B_CORE, h=H, w=W):
    half = h // 2
    sh = half + 2
    pw = w + 2
    ss = sh * pw  # slab partition stride (elements)
    nc = bacc.Bacc("TRN2", target_bir_lowering=False, debug=False, num_devices=N_CORES)
    x_in = nc.declare_dram_parameter("x", [n_img, CIN, h, w], F32, isOutput=False)
    wsgn_in = nc.declare_dram_parameter("wsgn", [128, 9 * 64], FP8, isOutput=False)
    scale_in = nc.declare_dram_parameter("scale", [128, 1], F32, isOutput=False)
    out_ext = nc.declare_dram_parameter("out", [n_img, COUT, h, w], BF16, isOutput=True)

    n_tiles = (half + RPT - 1) // RPT

    with tile.TileContext(nc) as tc, ExitStack() as ctx:
        wpool = ctx.enter_context(tc.tile_pool(name="wpool", bufs=1))
        spool = ctx.enter_context(tc.tile_pool(name="spool", bufs=1))
        xpool = ctx.enter_context(tc.tile_pool(name="xpool", bufs=4))
        ppool = ctx.enter_context(tc.tile_pool(name="ppool", bufs=4, space="PSUM"))
        opool = ctx.enter_context(tc.tile_pool(name="opool", bufs=2))

        wt2 = wpool.tile([128, 9 * 64], FP8, name="wt2")
        nc.sync.dma_start(wt2[:], wsgn_in[:])
        wt3 = wt2.rearrange("p (k m) -> p k m", m=64)
        sc = wpool.tile([128, 1], F32, name="sc")
        nc.sync.dma_start(sc[:], scale_in[:])

        # Two persistent slab buffers (manual ping-pong); pads zeroed once via
        # one contiguous memset each (strided byte memsets are ~4x slower and
        # serialized the first image's sign ops behind them).
        slabs = []
        for i in range(2):
            slab = spool.tile([128, sh * pw], FP8, name=f"slab{i}", tag=f"slab{i}")
            nc.vector.memset(slab[:], 0.0)
            slabs.append(slab)

        # staging: slab rows 1..half in chunks; leftovers done separately.
        # Finer chunks shorten the critical path to the image's first matmul.
        n_ch = 8 if half % 8 == 0 else (4 if half % 4 == 0 else 1)
        ch = half // n_ch  # slab rows per chunk (covers s=1..half)

        for img in range(n_img):
            slab = slabs[img % 2]
            s3 = slab.rearrange("p (r c) -> p r c", c=pw)

            # halo rows FIRST: bottom slab row 0 <- x row half-1 gates the very
            # first bottom-half tile, so it must not queue behind the big signs
            xs = xpool.tile([128, w], F32, name="xs", tag="xs")
            nc.sync.dma_start(xs[0:64, :], x_in[img, :, half : half + 1, :])
            nc.sync.dma_start(xs[64:128, :], x_in[img, :, half - 1 : half, :])
            nc.scalar.sign(s3[0:64, sh - 1, 1 : 1 + w], xs[0:64, :])
            nc.scalar.sign(s3[64:128, 0, 1 : 1 + w], xs[64:128, :])

            for c in range(n_ch):
                s_lo = 1 + c * ch  # slab row range [s_lo, s_lo+ch)
                xc = xpool.tile([128, ch * w], F32, name="xc", tag="xc")
                xc3 = xc.rearrange("p (r c) -> p r c", c=w)
                # top half: slab row s <- x row s-1
                nc.sync.dma_start(
                    xc[0:64, :], x_in[img, :, s_lo - 1 : s_lo - 1 + ch, :]
                )
                # bottom half: slab row s <- x row half - 1 + s
                nc.sync.dma_start(
                    xc[64:128, :],
                    x_in[img, :, half - 1 + s_lo : half - 1 + s_lo + ch, :],
                )
                nc.scalar.sign(s3[:, s_lo : s_lo + ch, 1 : 1 + w], xc3[:])

            # per-image batched output buffer: top half rows on partitions
            # 0:64, bottom half on 64:128 (cout is the fast partition index)
            obuf = opool.tile([128, half * w], BF16, name="obuf", tag="obuf")
            ob3 = obuf.rearrange("p (r c) -> p r c", c=w)

            for t in range(n_tiles):
                h0 = t * RPT
                R = min(RPT, half - h0)
                # DoubleRow requires col group 0 (DR + column tiling is
                # ISA-rejected), so each half keeps its own 64-partition psum.
                psumT = ppool.tile([64, R * w], F32, name="psumT", tag="psumT")
                psumB = ppool.tile([64, R * w], F32, name="psumB", tag="psumB")
                # issue T/B interleaved per schedule slot: the PE runs the two
                # row-group tiles concurrently only when their matmuls are
                # adjacent in program order (serial grouping costs ~2x).
                for k, (kind, wb, ro, co) in enumerate(SCHED):
                    for p0, psum in ((0, psumT), (64, psumB)):
                        if kind == "dr":
                            base = s3[p0 : p0 + 64, h0 + ro, co]
                            mov = AP(
                                tensor=base.tensor,
                                offset=base.offset,
                                ap=[[ss, 64], [pw, 2], [pw, R], [1, w]],
                            )
                            lhs = wt3[p0 : p0 + 64, wb : wb + 2, :]
                            pm = DR
                        else:
                            mov = s3[p0 : p0 + 64, h0 + ro : h0 + ro + R, co : co + w]
                            lhs = wt3[p0 : p0 + 64, wb, :]
                            pm = None
                        nc.tensor.matmul(
                            psum[:], lhs, mov,
                            start=(k == 0), stop=(k == len(SCHED) - 1),
                            perf_mode=pm,
                            tile_position=(p0, 0),
                        )
                # drains: DVE-only would be ~149us (64-partition ops); give
                # ACT ~2/13 of them (it has slack beside the 65us of sign)
                act_tile = (img * n_tiles + t) % 13 < 2
                eng = nc.scalar if act_tile else nc.vector
                if act_tile:
                    eng.mul(ob3[0:64, h0 : h0 + R, :], psumT[:], sc[0:64])
                    eng.mul(ob3[64:128, h0 : h0 + R, :], psumB[:], sc[0:64])
                else:
                    eng.tensor_scalar_mul(ob3[0:64, h0 : h0 + R, :], psumT[:], sc[0:64])
                    eng.tensor_scalar_mul(ob3[64:128, h0 : h0 + R, :], psumB[:], sc[0:64])
            # batched output: two ~1.6MB DMAs per image instead of 54 x 61KB
            nc.sync.dma_start(out_ext[img, :, 0:half, :], ob3[0:64])
            nc.sync.dma_start(out_ext[img, :, half:h, :], ob3[64:128])
    nc.finalize()
    return nc


_NC_CACHE = {}


def _get_nc():
    if "nc" not in _NC_CACHE:
        _NC_CACHE["nc"] = build_nc()
    return _NC_CACHE["nc"]


def _prep_weights(w):
    wc = np.clip(np.asarray(w, dtype=np.float32), -1.0, 1.0)
    scale = np.abs(wc).mean(axis=(1, 2, 3)).astype(np.float32).reshape(64, 1)
    s = np.sign(wc).astype(np.float32)  # [co, ci, kh, kw]
    buf = np.zeros((64, 9 * 64), dtype=np.float32)
    for b, (kh, kw) in enumerate(WBLOCKS):
        buf[:, b * 64 : b * 64 + 64] = s[:, :, kh, kw].T
    wsgn2 = np.concatenate([buf, buf], axis=0).astype(ml_dtypes.float8_e4m3)
    return wsgn2, np.concatenate([scale, scale], axis=0)


def kernel(x, w, _trace=False):
    x = np.ascontiguousarray(np.asarray(x, dtype=np.float32))
    wsgn2, scale = _prep_weights(w)
    nc = _get_nc()
    in_maps = [
        {"x": x[i * B_CORE : (i + 1) * B_CORE], "wsgn": wsgn2, "scale": scale}
        for i in range(N_CORES)
    ]
    # The axon-proxied execution occasionally faults with a transient
    # NRT_EXEC_UNIT_UNRECOVERABLE; a retry on a fresh session recovers.
    last_err = None
    for attempt in range(3):
        try:
            res = run_bass_kernel_spmd(nc, in_maps, list(range(N_CORES)), trace=_trace)
            break
        except Exception as e:  # noqa: BLE001
            last_err = e
            import time as _time
            _time.sleep(3.0)
    else:
        raise last_err
    out = np.concatenate(
        [res.results[i]["out"].astype(np.float32) for i in range(N_CORES)], axis=0
    )
    if _trace:
        return out, res
    return out


# revision 21
# speedup vs baseline: 1.0133x; 1.0133x over previous
"""BinaryConv2d Trainium2 kernel (8-core batch-parallel, fp8 DoubleRow PE).

Per image: top half rows on partitions 0:64, bottom half on 64:128 (each a
zero-padded fp8 slab of HALF+2 rows; sign(x) is exactly representable in
fp8e4). Each 3x3 conv runs as FIVE DoubleRow matmuls per PSUM tile (two
kernel positions contracted per matmul, 2 fp8 elems/partition/cycle), issued
concurrently on PE row groups (tile_position (0,0) and (64,0)). Output is
scaled on DVE into bf16 (integer counts <=576 keep full precision at the
needed tolerance) halving output DMA; host upcasts to fp32.
"""
import sys
import numpy as np
from contextlib import ExitStack

sys.path.insert(0, "/root/.axon_site/_ro/trn_rl_repo")
sys.path.insert(0, "/opt/trn_rl_repo")

import ml_dtypes
import concourse.bass as bass
import concourse.bacc as bacc
import concourse.mybir as mybir
import concourse.tile as tile
from concourse.ap import AP
from concourse.bass_utils import run_bass_kernel_spmd

F32 = mybir.dt.float32
BF16 = mybir.dt.bfloat16
FP8 = mybir.dt.float8e4
# A DoubleRow matmul with start=False,stop=False (middle of an accumulation
# group) hard-faults the exec unit, and DoubleRowSwInterleave fails walrus's
# ldweights ISA check. So DR pairs are legal only as the start and stop
# matmuls of a group; the other 5 taps run as regular-fp8 singles.
DR = mybir.MatmulPerfMode.DoubleRow

N_CORES = 8
B, CIN, COUT, KS = 32, 64, 64, 3
H = W = 160
B_CORE = B // N_CORES
HALF = H // 2          # rows per half
SH = HALF + 2          # slab rows per half (1 halo/pad row each side)
PW = W + 2
RPT = 3                # output rows per PSUM tile

# Per-PSUM-tile schedule: 7 matmuls. DR pairs ((0,0),(1,0)) and ((0,2),(1,2))
# at the start/stop slots; taps (0,1),(1,1),(2,1),(2,0),(2,2) as regular-fp8
# singles in between. Entries: (weight offset blocks, (row, col) or
# (row, col, pairstride) for DR).
# Weight free-dim layout (64-wide blocks):
#   [0]=(0,0) [1]=(1,0) [2]=(0,1) [3]=(1,1) [4]=(2,1) [5]=(2,0) [6]=(2,2)
#   [7]=(0,2) [8]=(1,2)
SCHED = [
    ("dr", 0, 0, 0),     # pair blocks 0,1: taps (0,0),(1,0); mov base (h0, 0)
    ("s", 2, 0, 1),      # (0,1)
    ("s", 3, 1, 1),      # (1,1)
    ("s", 4, 2, 1),      # (2,1)
    ("s", 5, 2, 0),      # (2,0)
    ("s", 6, 2, 2),      # (2,2)
    ("dr", 7, 0, 2),     # pair blocks 7,8: taps (0,2),(1,2); mov base (h0, 2)
]
WBLOCKS = [
    (0, 0), (1, 0), (0, 1), (1, 1), (2, 1), (2, 0), (2, 2), (0, 2), (1, 2)
]


def build_nc(n_img=B_CORE, h=H, w=W):
    half = h // 2
    sh = half + 2
    pw = w + 2
    ss = sh * pw  # slab partition stride (elements)
    nc = bacc.Bacc("TRN2", target_bir_lowering=False, debug=False, num_devices=N_CORES)
    x_in = nc.declare_dram_parameter("x", [n_img, CIN, h, w], F32, isOutput=False)
    wsgn_in = nc.declare_dram_parameter("wsgn", [128, 9 * 64], FP8, isOutput=False)
    scale_in = nc.declare_dram_parameter("scale", [128, 1], F32, isOutput=False)
    out_ext = nc.declare_dram_parameter("out", [n_img, COUT, h, w], BF16, isOutput=True)

    n_tiles = (half + RPT - 1) // RPT

    with tile.TileContext(nc) as tc, ExitStack() as ctx:
        wpool = ctx.enter_context(tc.tile_pool(name="wpool", bufs=1))
        spool = ctx.enter_context(tc.tile_pool(name="spool", bufs=1))
        xpool = ctx.enter_context(tc.tile_pool(name="xpool", bufs=3))
        ppool = ctx.enter_context(tc.tile_pool(name="ppool", bufs=4, space="PSUM"))
        opool = ctx.enter_context(tc.tile_pool(name="opool", bufs=2))

        wt2 = wpool.tile([128, 9 * 64], FP8, name="wt2")
        nc.sync.dma_start(wt2[:], wsgn_in[:])
        wt3 = wt2.rearrange("p (k m) -> p k m", m=64)
        sc = wpool.tile([128, 1], F32, name="sc")
        nc.sync.dma_start(sc[:], scale_in[:])

        # Two persistent slab buffers (manual ping-pong); pads zeroed once.
        slabs = []
        for i in range(2):
            slab = spool.tile([128, sh * pw], FP8, name=f"slab{i}", tag=f"slab{i}")
            s3 = slab.rearrange("p (r c) -> p r c", c=pw)
            # col pads: elements r*pw + {0, pw-1} for all slab rows
            nc.vector.memset(slab[:, 0 : (sh - 1) * pw + pw : pw], 0.0)
            nc.vector.memset(slab[:, pw - 1 : sh * pw : pw], 0.0)
            # row pads: top half row 0 (partitions 0:64), bottom half last row
            nc.vector.memset(s3[0:64, 0, :], 0.0)
            nc.vector.memset(s3[64:128, sh - 1, :], 0.0)
            slabs.append(slab)

        # staging: slab rows 1..half in chunks; leftovers done separately.
        # Finer chunks shorten the critical path to the image's first matmul.
        n_ch = 4 if half % 4 == 0 else (2 if half % 2 == 0 else 1)
        ch = half // n_ch  # slab rows per chunk (covers s=1..half)

        for img in range(n_img):
            slab = slabs[img % 2]
            s3 = slab.rearrange("p (r c) -> p r c", c=pw)

            # halo rows FIRST: bottom slab row 0 <- x row half-1 gates the very
            # first bottom-half tile, so it must not queue behind the big signs
            xs = xpool.tile([128, w], F32, name="xs", tag="xs")
            nc.sync.dma_start(xs[0:64, :], x_in[img, :, half : half + 1, :])
            nc.sync.dma_start(xs[64:128, :], x_in[img, :, half - 1 : half, :])
            nc.scalar.sign(s3[0:64, sh - 1, 1 : 1 + w], xs[0:64, :])
            nc.scalar.sign(s3[64:128, 0, 1 : 1 + w], xs[64:128, :])

            for c in range(n_ch):
                s_lo = 1 + c * ch  # slab row range [s_lo, s_lo+ch)
                xc = xpool.tile([128, ch * w], F32, name="xc", tag="xc")
                xc3 = xc.rearrange("p (r c) -> p r c", c=w)
                # top half: slab row s <- x row s-1
                nc.sync.dma_start(
                    xc[0:64, :], x_in[img, :, s_lo - 1 : s_lo - 1 + ch, :]
                )
                # bottom half: slab row s <- x row half - 1 + s
                nc.sync.dma_start(
                    xc[64:128, :],
                    x_in[img, :, half - 1 + s_lo : half - 1 + s_lo + ch, :],
                )
                nc.scalar.sign(s3[:, s_lo : s_lo + ch, 1 : 1 + w], xc3[:])

            # per-image batched output buffer: top half rows on partitions
            # 0:64, bottom half on 64:128 (cout is the fast partition index)
            obuf = opool.tile([128, half * w], BF16, name="obuf", tag="obuf")
            ob3 = obuf.rearrange("p (r c) -> p r c", c=w)

            for t in range(n_tiles):
                h0 = t * RPT
                R = min(RPT, half - h0)
                # DoubleRow requires col group 0 (DR + column tiling is
                # ISA-rejected), so each half keeps its own 64-partition psum.
                psumT = ppool.tile([64, R * w], F32, name="psumT", tag="psumT")
                psumB = ppool.tile([64, R * w], F32, name="psumB", tag="psumB")
                # issue T/B interleaved per schedule slot: the PE runs the two
                # row-group tiles concurrently only when their matmuls are
                # adjacent in program order (serial grouping costs ~2x).
                for k, (kind, wb, ro, co) in enumerate(SCHED):
                    for p0, psum in ((0, psumT), (64, psumB)):
                        if kind == "dr":
                            base = s3[p0 : p0 + 64, h0 + ro, co]
                            mov = AP(
                                tensor=base.tensor,
                                offset=base.offset,
                                ap=[[ss, 64], [pw, 2], [pw, R], [1, w]],
                            )
                            lhs = wt3[p0 : p0 + 64, wb : wb + 2, :]
                            pm = DR
                        else:
                            mov = s3[p0 : p0 + 64, h0 + ro : h0 + ro + R, co : co + w]
                            lhs = wt3[p0 : p0 + 64, wb, :]
                            pm = None
                        nc.tensor.matmul(
                            psum[:], lhs, mov,
                            start=(k == 0), stop=(k == len(SCHED) - 1),
                            perf_mode=pm,
                            tile_position=(p0, 0),
                        )
                # drains: DVE-only would be ~149us (64-partition ops); give
                # ACT ~2/13 of them (it has slack beside the 65us of sign)
                act_tile = (img * n_tiles + t) % 13 < 2
                eng = nc.scalar if act_tile else nc.vector
                if act_tile:
                    eng.mul(ob3[0:64, h0 : h0 + R, :], psumT[:], sc[0:64])
                    eng.mul(ob3[64:128, h0 : h0 + R, :], psumB[:], sc[0:64])
                else:
                    eng.tensor_scalar_mul(ob3[0:64, h0 : h0 + R, :], psumT[:], sc[0:64])
                    eng.tensor_scalar_mul(ob3[64:128, h0 : h0 + R, :], psumB[:], sc[0:64])
            # batched output: two ~1.6MB DMAs per image instead of 54 x 61KB
            nc.sync.dma_start(out_ext[img, :, 0:half, :], ob3[0:64])
            nc.sync.dma_start(out_ext[img, :, half:h, :], ob3[64:128])
    nc.finalize()
    return nc


_NC_CACHE = {}


def _get_nc():
    if "nc" not in _NC_CACHE:
        _NC_CACHE["nc"] = build_nc()
    return _NC_CACHE["nc"]


def _prep_weights(w):
    wc = np.clip(np.asarray(w, dtype=np.float32), -1.0, 1.0)
    scale = np.abs(wc).mean(axis=(1, 2, 3)).astype(np.float32).reshape(64, 1)
    s = np.sign(wc).astype(np.float32)  # [co, ci, kh, kw]
    buf = np.zeros((64, 9 * 64), dtype=np.float32)
    for b, (kh, kw) in enumerate(WBLOCKS):
        buf[:, b * 64 : b * 64 + 64] = s[:, :, kh, kw].T
    wsgn2 = np.concatenate([buf, buf], axis=0).astype(ml_dtypes.float8_e4m3)
    return wsgn2, np.concatenate([scale, scale], axis=0)


def kernel(x, w, _trace=False):
    x = np.ascontiguousarray(np.asarray(x, dtype=np.float32))
    wsgn2, scale = _prep_weights(w)
    nc = _get_nc()
    in_maps = [
        {"x": x[i * B_CORE : (i + 1) * B_CORE], "wsgn": wsgn2, "scale": scale}
        for i in range(N_CORES)
    ]
    # The axon-proxied execution occasionally faults with a transient
    # NRT_EXEC_UNIT_UNRECOVERABLE; a retry on a fresh session recovers.
    last_err = None
    for attempt in range(3):
        try:
            res = run_bass_kernel_spmd(nc, in_maps, list(range(N_CORES)), trace=_trace)
            break
        except Exception as e:  # noqa: BLE001
            last_err = e
            import time as _time
            _time.sleep(3.0)
    else:
        raise last_err
    out = np.concatenate(
        [res.results[i]["out"].astype(np.float32) for i in range(N_CORES)], axis=0
    )
    if _trace:
        return out, res
    return out


# revision 25
# speedup vs baseline: 1.1122x; 1.0977x over previous
"""BinaryConv2d Trainium2 kernel (8-core batch-parallel, fp8 DoubleRow PE).

Per image: top half rows on partitions 0:64, bottom half on 64:128 (each a
zero-padded fp8 slab of HALF+2 rows; sign(x) is exactly representable in
fp8e4). Each 3x3 conv runs as FIVE DoubleRow matmuls per PSUM tile (two
kernel positions contracted per matmul, 2 fp8 elems/partition/cycle), issued
concurrently on PE row groups (tile_position (0,0) and (64,0)). Output is
scaled on DVE into bf16 (integer counts <=576 keep full precision at the
needed tolerance) halving output DMA; host upcasts to fp32.
"""
import sys
import numpy as np
from contextlib import ExitStack

sys.path.insert(0, "/root/.axon_site/_ro/trn_rl_repo")
sys.path.insert(0, "/opt/trn_rl_repo")

import ml_dtypes
import concourse.bass as bass
import concourse.bacc as bacc
import concourse.mybir as mybir
import concourse.tile as tile
from concourse.ap import AP
from concourse.bass_utils import run_bass_kernel_spmd

F32 = mybir.dt.float32
BF16 = mybir.dt.bfloat16
FP8 = mybir.dt.float8e4
# A DoubleRow matmul with start=False,stop=False (middle of an accumulation
# group) hard-faults the exec unit, and DoubleRowSwInterleave fails walrus's
# ldweights ISA check. So DR pairs are legal only as the start and stop
# matmuls of a group; the other 5 taps run as regular-fp8 singles.
DR = mybir.MatmulPerfMode.DoubleRow

N_CORES = 8
B, CIN, COUT, KS = 32, 64, 64, 3
H = W = 160
B_CORE = B // N_CORES
HALF = H // 2          # rows per half
SH = HALF + 2          # slab rows per half (1 halo/pad row each side)
PW = W + 2
RPT = 3                # output rows per PSUM tile

# Per-PSUM-tile schedule: 7 matmuls. DR pairs ((0,0),(1,0)) and ((0,2),(1,2))
# at the start/stop slots; taps (0,1),(1,1),(2,1),(2,0),(2,2) as regular-fp8
# singles in between. Entries: (weight offset blocks, (row, col) or
# (row, col, pairstride) for DR).
# Weight free-dim layout (64-wide blocks):
#   [0]=(0,0) [1]=(1,0) [2]=(0,1) [3]=(1,1) [4]=(2,1) [5]=(2,0) [6]=(2,2)
#   [7]=(0,2) [8]=(1,2)
SCHED = [
    ("dr", 0, 0, 0),     # pair blocks 0,1: taps (0,0),(1,0); mov base (h0, 0)
    ("s", 2, 0, 1),      # (0,1)
    ("s", 3, 1, 1),      # (1,1)
    ("s", 4, 2, 1),      # (2,1)
    ("s", 5, 2, 0),      # (2,0)
    ("s", 6, 2, 2),      # (2,2)
    ("dr", 7, 0, 2),     # pair blocks 7,8: taps (0,2),(1,2); mov base (h0, 2)
]
WBLOCKS = [
    (0, 0), (1, 0), (0, 1), (1, 1), (2, 1), (2, 0), (2, 2), (0, 2), (1, 2)
]
# output row blocks, keyed by the PSUM tile index after which rows [r0,r1) of
# each half are fully drained (27 tiles x 3 rows = 80 rows per half)
OBLOCKS = {13: (0, 42), 22: (42, 69), 26: (69, 80)}


def build_nc(n_img=B_CORE, h=H, w=W):
    half = h // 2
    sh = half + 2
    pw = w + 2
    ss = sh * pw  # slab partition stride (elements)
    nc = bacc.Bacc("TRN2", target_bir_lowering=False, debug=False, num_devices=N_CORES)
    x_in = nc.declare_dram_parameter("x", [n_img, CIN, h, w], F32, isOutput=False)
    wsgn_in = nc.declare_dram_parameter("wsgn", [128, 9 * 64], FP8, isOutput=False)
    scale_in = nc.declare_dram_parameter("scale", [128, 1], F32, isOutput=False)
    out_ext = nc.declare_dram_parameter("out", [n_img, COUT, h, w], BF16, isOutput=True)

    n_tiles = (half + RPT - 1) // RPT

    with tile.TileContext(nc) as tc, ExitStack() as ctx:
        wpool = ctx.enter_context(tc.tile_pool(name="wpool", bufs=1))
        spool = ctx.enter_context(tc.tile_pool(name="spool", bufs=1))
        xpool = ctx.enter_context(tc.tile_pool(name="xpool", bufs=3))
        ppool = ctx.enter_context(tc.tile_pool(name="ppool", bufs=4, space="PSUM"))
        opool = ctx.enter_context(tc.tile_pool(name="opool", bufs=2))

        wt2 = wpool.tile([128, 9 * 64], FP8, name="wt2")
        nc.sync.dma_start(wt2[:], wsgn_in[:])
        wt3 = wt2.rearrange("p (k m) -> p k m", m=64)
        sc = wpool.tile([128, 1], F32, name="sc")
        nc.sync.dma_start(sc[:], scale_in[:])

        # Two persistent slab buffers (manual ping-pong); pads zeroed once.
        slabs = []
        for i in range(2):
            slab = spool.tile([128, sh * pw], FP8, name=f"slab{i}", tag=f"slab{i}")
            s3 = slab.rearrange("p (r c) -> p r c", c=pw)
            # col pads: elements r*pw + {0, pw-1} for all slab rows
            nc.vector.memset(slab[:, 0 : (sh - 1) * pw + pw : pw], 0.0)
            nc.vector.memset(slab[:, pw - 1 : sh * pw : pw], 0.0)
            # row pads: top half row 0 (partitions 0:64), bottom half last row
            nc.vector.memset(s3[0:64, 0, :], 0.0)
            nc.vector.memset(s3[64:128, sh - 1, :], 0.0)
            slabs.append(slab)

        # staging: slab rows 1..half in chunks; leftovers done separately.
        # A small first chunk shortens the critical path to the image's first
        # matmul (it only needs slab rows 0..RPT+1); the rest in big chunks.
        chunks = [6, 18, 18, 19, 19] if half == 80 else [half]
        assert sum(chunks) == half

        for img in range(n_img):
            slab = slabs[img % 2]
            s3 = slab.rearrange("p (r c) -> p r c", c=pw)

            # halo rows FIRST: bottom slab row 0 <- x row half-1 gates the very
            # first bottom-half tile, so it must not queue behind the big signs
            xs = xpool.tile([128, w], F32, name="xs", tag="xs")
            nc.sync.dma_start(xs[0:64, :], x_in[img, :, half : half + 1, :])
            nc.sync.dma_start(xs[64:128, :], x_in[img, :, half - 1 : half, :])
            nc.scalar.sign(s3[0:64, sh - 1, 1 : 1 + w], xs[0:64, :])
            nc.scalar.sign(s3[64:128, 0, 1 : 1 + w], xs[64:128, :])

            s_lo = 1
            for ch in chunks:
                # slab row range [s_lo, s_lo+ch)
                xc = xpool.tile([128, ch * w], F32, name="xc", tag=f"xc{ch}")
                xc3 = xc.rearrange("p (r c) -> p r c", c=w)
                # top half: slab row s <- x row s-1
                nc.sync.dma_start(
                    xc[0:64, :], x_in[img, :, s_lo - 1 : s_lo - 1 + ch, :]
                )
                # bottom half: slab row s <- x row half - 1 + s
                nc.sync.dma_start(
                    xc[64:128, :],
                    x_in[img, :, half - 1 + s_lo : half - 1 + s_lo + ch, :],
                )
                nc.scalar.sign(s3[:, s_lo : s_lo + ch, 1 : 1 + w], xc3[:])
                s_lo += ch

            # per-image batched output buffer: top half rows on partitions
            # 0:64, bottom half on 64:128 (cout is the fast partition index)
            obuf = opool.tile([128, half * w], BF16, name="obuf", tag="obuf")
            ob3 = obuf.rearrange("p (r c) -> p r c", c=w)

            for t in range(n_tiles):
                h0 = t * RPT
                R = min(RPT, half - h0)
                # DoubleRow requires col group 0 (DR + column tiling is
                # ISA-rejected), so each half keeps its own 64-partition psum.
                psumT = ppool.tile([64, R * w], F32, name="psumT", tag="psumT")
                psumB = ppool.tile([64, R * w], F32, name="psumB", tag="psumB")
                # issue T/B interleaved per schedule slot: the PE runs the two
                # row-group tiles concurrently only when their matmuls are
                # adjacent in program order (serial grouping costs ~2x).
                for k, (kind, wb, ro, co) in enumerate(SCHED):
                    for p0, psum in ((0, psumT), (64, psumB)):
                        if kind == "dr":
                            base = s3[p0 : p0 + 64, h0 + ro, co]
                            mov = AP(
                                tensor=base.tensor,
                                offset=base.offset,
                                ap=[[ss, 64], [pw, 2], [pw, R], [1, w]],
                            )
                            lhs = wt3[p0 : p0 + 64, wb : wb + 2, :]
                            pm = DR
                        else:
                            mov = s3[p0 : p0 + 64, h0 + ro : h0 + ro + R, co : co + w]
                            lhs = wt3[p0 : p0 + 64, wb, :]
                            pm = None
                        nc.tensor.matmul(
                            psum[:], lhs, mov,
                            start=(k == 0), stop=(k == len(SCHED) - 1),
                            perf_mode=pm,
                            tile_position=(p0, 0),
                        )
                # drains: DVE-only would be ~149us (64-partition ops); give
                # ACT ~2/13 of them (it has slack beside the 65us of sign)
                act_tile = (img * n_tiles + t) % 13 < 2
                eng = nc.scalar if act_tile else nc.vector
                if act_tile:
                    eng.mul(ob3[0:64, h0 : h0 + R, :], psumT[:], sc[0:64])
                    eng.mul(ob3[64:128, h0 : h0 + R, :], psumB[:], sc[0:64])
                else:
                    eng.tensor_scalar_mul(ob3[0:64, h0 : h0 + R, :], psumT[:], sc[0:64])
                    eng.tensor_scalar_mul(ob3[64:128, h0 : h0 + R, :], psumB[:], sc[0:64])
                # blocked output (~0.4-0.9MB per DMA keeps efficiency while the
                # small final block shrinks the non-overlappable tail wait)
                if t in OBLOCKS:
                    r0, r1 = OBLOCKS[t]
                    nc.sync.dma_start(
                        out_ext[img, :, r0:r1, :], ob3[0:64, r0:r1, :]
                    )
                    nc.sync.dma_start(
                        out_ext[img, :, half + r0 : half + r1, :],
                        ob3[64:128, r0:r1, :],
                    )
    nc.finalize()
    return nc


_NC_CACHE = {}


def _get_nc():
    if "nc" not in _NC_CACHE:
        _NC_CACHE["nc"] = build_nc()
    return _NC_CACHE["nc"]


def _prep_weights(w):
    wc = np.clip(np.asarray(w, dtype=np.float32), -1.0, 1.0)
    scale = np.abs(wc).mean(axis=(1, 2, 3)).astype(np.float32).reshape(64, 1)
    s = np.sign(wc).astype(np.float32)  # [co, ci, kh, kw]
    buf = np.zeros((64, 9 * 64), dtype=np.float32)
    for b, (kh, kw) in enumerate(WBLOCKS):
        buf[:, b * 64 : b * 64 + 64] = s[:, :, kh, kw].T
    wsgn2 = np.concatenate([buf, buf], axis=0).astype(ml_dtypes.float8_e4m3)
    return wsgn2, np.concatenate([scale, scale], axis=0)


def kernel(x, w, _trace=False):
    x = np.ascontiguousarray(np.asarray(x, dtype=np.float32))
    wsgn2, scale = _prep_weights(w)
    nc = _get_nc()
    in_maps = [
        {"x": x[i * B_CORE : (i + 1) * B_CORE], "wsgn": wsgn2, "scale": scale}
        for i in range(N_CORES)
    ]
    # The axon-proxied execution occasionally faults with a transient
    # NRT_EXEC_UNIT_UNRECOVERABLE; a retry on a fresh session recovers.
    last_err = None
    for attempt in range(3):
        try:
            res = run_bass_kernel_spmd(nc, in_maps, list(range(N_CORES)), trace=_trace)
            break
        except Exception as e:  # noqa: BLE001
            last_err = e
            import time as _time
            _time.sleep(3.0)
    else:
        raise last_err
    out = np.concatenate(
        [res.results[i]["out"].astype(np.float32) for i in range(N_CORES)], axis=0
    )
    if _trace:
        return out, res
    return out
